# revision 1
# baseline (speedup 1.0000x reference)
"""Distributed memory-shard scale kernel for Trainium2 (8 NeuronCores).

Computes out[b, s, d] = x[b, s, d] * shards[shard_map[d], d] for
x: [4, 4096, 4096] f32, shards: [8, 4096] f32, shard_map: [4096] int.

Strategy: data-parallel over the flattened (batch*seq) rows — each of the
8 cores owns a contiguous 2048-row slice of x and replicates the tiny
shards/shard_map inputs. On device each core:
  1. builds w[d] = shards[shard_map[d], d] with masked multiply-accumulate
     over the 8 shard rows, 256 dims per partition on 16 partitions,
  2. flattens w onto one partition with a single SBUF→SBUF DMA and
     replicates it to all 128 partitions with K=1 outer-product matmuls
     (PE + DVE only — no broadcast DMA traffic),
  3. streams its x slice through SBUF in [128, 4096] tiles, multiplying by
     the replicated weight row and writing back out. The first row block
     is processed in quarter-width chunks so the store stream starts as
     early as possible.
"""

import numpy as np

import bass_rust as _bass_rust
import concourse.bass as bass
import concourse.tile as tile
from concourse import mybir
from concourse.bass_utils import run_bass_kernel_spmd

N_CORES = 8
BATCH, SEQ, DIM = 4, 4096, 4096
NUM_SHARDS = 8
ROWS_TOTAL = BATCH * SEQ               # 16384
ROWS_PER_CORE = ROWS_TOTAL // N_CORES  # 2048
P = 128                                # SBUF partitions
N_TILES = ROWS_PER_CORE // P           # 16
WP = 16                                # partitions used by the w build
DPW = DIM // WP                        # dims per partition in w build (256)
AUX_W = (1 + NUM_SHARDS) * DPW         # aux free width (2304)

TRACE = False       # set True (e.g. from test.py) to capture an NTFF profile
LAST_RESULT = None  # BassKernelResults of the most recent kernel() call

_cached_nc = None


def _build_program() -> bass.Bass:
    f32 = mybir.dt.float32
    nc = bass.Bass()
    x_in = nc.dram_tensor("x", [ROWS_PER_CORE, DIM], f32, kind="ExternalInput")
    # aux packs shard_map and shards into one [16, 2304] tensor:
    #   aux[p, 0:DPW]         = shard_map[p*DPW : (p+1)*DPW]  (as f32)
    #   aux[p, (1+s)*DPW + j] = shards[s, p*DPW + j]
    aux_in = nc.dram_tensor("aux", [WP, AUX_W], f32, kind="ExternalInput")
    out = nc.dram_tensor("out", [ROWS_PER_CORE, DIM], f32,
                         kind="ExternalOutput")

    with tile.TileContext(nc) as tc:
        with tc.tile_pool(name="const", bufs=1) as cpool, \
             tc.tile_pool(name="xp", bufs=5) as xpool:
            # ones row for the broadcast matmuls — engine op, no DMA
            ones = cpool.tile([1, P], f32)
            nc.vector.memset(ones[:], 1.0)
            # --- one-time: w[d] = shards[shard_map[d], d], [16, 256] ---
            auxt = cpool.tile([WP, AUX_W], f32)
            nc.sync.dma_start(auxt[:], aux_in[:])
            mf = auxt[:, 0:DPW]
            wacc = cpool.tile([WP, DPW], f32)
            tmp = cpool.tile([WP, DPW], f32)
            nc.vector.memset(wacc[:], 0.0)
            for s in range(NUM_SHARDS):
                # tmp = (shard_map == s) * shards[s, :]
                nc.vector.scalar_tensor_tensor(
                    out=tmp[:], in0=mf, scalar=float(s),
                    in1=auxt[:, (1 + s) * DPW:(2 + s) * DPW],
                    op0=mybir.AluOpType.is_equal, op1=mybir.AluOpType.mult)
                nc.vector.tensor_add(wacc[:], wacc[:], tmp[:])

            # --- flatten w onto one partition (single SB→SB DMA on the
            # idle ACT ring), then replicate to all 128 partitions with
            # K=1 outer-product matmuls ones[1,128].T @ wrow[1,512] →
            # PSUM[128,512]; PE+DVE only.
            # stage the flattened w row in w128's partition 0 — the
            # broadcast copies later overwrite it with identical values,
            # and dropping the separate tile frees 16KB of SBUF width
            # for a 5th x buffer
            w128 = cpool.tile([P, DIM], f32)
            wrow = w128[0:1, :]
            nc.scalar.dma_start(wrow, wacc[:])
            MMF = 512  # one PSUM bank per matmul
            with tc.tile_pool(name="ps", bufs=8, space="PSUM") as ppool:
                for k in range(DIM // MMF):
                    mm = ppool.tile([P, MMF], f32)
                    nc.tensor.matmul(mm[:], ones[:],
                                     w128[0:1, k * MMF:(k + 1) * MMF],
                                     start=True, stop=True)
                    nc.vector.tensor_copy(w128[:, k * MMF:(k + 1) * MMF],
                                          mm[:])

            # --- stream x through SBUF, scaling by w ---
            # [128, 8192] tiles: each partition holds two consecutive x
            # rows (32KB contiguous per partition -> bigger DMA
            # descriptors, less per-packet overhead). Column half t of a
            # tile is row 2p+t, so each half multiplies against w128
            # directly. First tile runs in quarter chunks so the store
            # stream starts early; last tile stores in quarters to
            # shorten the final mul->store chain.
            x2v = x_in.rearrange("(i p t) d -> i p (t d)", p=P, t=2)
            o2v = out.rearrange("(i p t) d -> i p (t d)", p=P, t=2)
            N_BIG = ROWS_PER_CORE // (2 * P)   # 8
            QW = DIM // 2                      # quarter width (2048)
            for i in range(N_BIG):
                xt = xpool.tile([P, 2 * DIM], f32)
                if i == 0:
                    for q in range(4):
                        cols = slice(q * QW, (q + 1) * QW)
                        nc.sync.dma_start(xt[:, cols], x2v[i, :, cols])
                        nc.vector.tensor_mul(
                            xt[:, cols], xt[:, cols],
                            w128[:, (q % 2) * QW:(q % 2 + 1) * QW])
                        nc.scalar.dma_start(o2v[i, :, cols], xt[:, cols])
                else:
                    nc.sync.dma_start(xt[:], x2v[i])
                    for h in range(2):
                        cols = slice(h * DIM, (h + 1) * DIM)
                        nc.vector.tensor_mul(xt[:, cols], xt[:, cols],
                                             w128[:])
                    if i < N_BIG - 1:
                        nc.scalar.dma_start(o2v[i], xt[:])
                    else:
                        for q in range(4):
                            cols = slice(q * QW, (q + 1) * QW)
                            nc.scalar.dma_start(o2v[i, :, cols],
                                                xt[:, cols])
    # TRN2 allows one sync wait per instruction; split multi-wait
    # instructions the way bacc's compile pipeline does.
    _bass_rust.generate_event_semaphores(nc)
    return nc


def _marshal(shards: np.ndarray, shard_map: np.ndarray):
    sh = np.asarray(shards, dtype=np.float32)
    aux = np.empty((WP, AUX_W), dtype=np.float32)
    aux[:, 0:DPW] = np.asarray(shard_map).astype(np.float32).reshape(WP, DPW)
    # aux[p, (1+s)*DPW + j] = shards[s, p*DPW + j]
    aux[:, DPW:] = sh.reshape(NUM_SHARDS, WP, DPW).transpose(
        1, 0, 2).reshape(WP, NUM_SHARDS * DPW)
    return aux


def kernel(x, shards, shard_map):
    global _cached_nc, LAST_RESULT
    if _cached_nc is None:
        _cached_nc = _build_program()
    nc = _cached_nc

    x2 = np.asarray(x, dtype=np.float32).reshape(ROWS_TOTAL, DIM)
    aux = _marshal(shards, shard_map)

    in_maps = [
        {"x": x2[c * ROWS_PER_CORE:(c + 1) * ROWS_PER_CORE], "aux": aux}
        for c in range(N_CORES)
    ]
    res = run_bass_kernel_spmd(nc, in_maps, core_ids=list(range(N_CORES)),
                               trace=TRACE)
    LAST_RESULT = res
    return np.concatenate([r["out"] for r in res.results],
                          axis=0).reshape(BATCH, SEQ, DIM)



# revision 2
# speedup vs baseline: 1.5241x; 1.5241x over previous
"""Distributed memory-shard scale kernel for Trainium2 (8 NeuronCores).

Computes out[b, s, d] = x[b, s, d] * shards[shard_map[d], d] for
x: [4, 4096, 4096] f32, shards: [8, 4096] f32, shard_map: [4096] int.

Strategy: data-parallel over the flattened (batch*seq) rows — each of the
8 cores owns a contiguous 2048-row slice of x and replicates the tiny
shards/shard_map inputs. The kernel is HBM-bandwidth-bound (pure
elementwise scale), so the x stream is staged in bf16: the host casts x
to bf16, the device streams bf16 tiles, multiplies by the bf16 weight
row on DVE (2x 16-bit throughput), stores bf16, and the host upcasts the
result to f32. This halves HBM traffic (64MB -> 32MB per core) for a
~1.9e-3 relative error, well inside the 2e-2 budget. On device each core:
  1. builds w[d] = shards[shard_map[d], d] in f32 with masked
     multiply-accumulate over the 8 shard rows (16 partitions x 256 dims),
     then rounds to bf16,
  2. flattens w onto one partition with a single SBUF->SBUF DMA and
     replicates it to all 128 partitions with K=1 outer-product matmuls
     (PE + DVE only — no broadcast DMA traffic),
  3. streams its x slice through SBUF in [128, 2*4096] bf16 tiles,
     multiplying by the replicated weight row and writing back out. The
     first row block is processed in quarter-width chunks so the store
     stream starts as early as possible.
"""

import numpy as np
import ml_dtypes

import bass_rust as _bass_rust
import concourse.bass as bass
import concourse.tile as tile
from concourse import mybir
from concourse.bass_utils import run_bass_kernel_spmd

N_CORES = 8
BATCH, SEQ, DIM = 4, 4096, 4096
NUM_SHARDS = 8
ROWS_TOTAL = BATCH * SEQ               # 16384
ROWS_PER_CORE = ROWS_TOTAL // N_CORES  # 2048
P = 128                                # SBUF partitions
WP = 16                                # partitions used by the w build
DPW = DIM // WP                        # dims per partition in w build (256)
AUX_W = (1 + NUM_SHARDS) * DPW         # aux free width (2304)

BF16 = ml_dtypes.bfloat16

TRACE = False       # set True (e.g. from test.py) to capture an NTFF profile
LAST_RESULT = None  # BassKernelResults of the most recent kernel() call

_cached_nc = None


def _build_program() -> bass.Bass:
    f32 = mybir.dt.float32
    bf16 = mybir.dt.bfloat16
    nc = bass.Bass()
    x_in = nc.dram_tensor("x", [ROWS_PER_CORE, DIM], bf16, kind="ExternalInput")
    # aux packs shard_map and shards into one [16, 2304] f32 tensor:
    #   aux[p, 0:DPW]         = shard_map[p*DPW : (p+1)*DPW]  (as f32)
    #   aux[p, (1+s)*DPW + j] = shards[s, p*DPW + j]
    aux_in = nc.dram_tensor("aux", [WP, AUX_W], f32, kind="ExternalInput")
    out = nc.dram_tensor("out", [ROWS_PER_CORE, DIM], bf16,
                         kind="ExternalOutput")

    with tile.TileContext(nc) as tc:
        with tc.tile_pool(name="const", bufs=1) as cpool, \
             tc.tile_pool(name="xp", bufs=8) as xpool:
            # ones row for the broadcast matmuls — engine op, no DMA
            ones = cpool.tile([1, P], bf16)
            nc.vector.memset(ones[:], 1.0)
            # --- one-time: w[d] = shards[shard_map[d], d], [16, 256] f32.
            # aux load + flatten ride the ACT HWDGE ring so the sync ring
            # can start streaming x tiles at t=0.
            auxt = cpool.tile([WP, AUX_W], f32)
            nc.scalar.dma_start(auxt[:], aux_in[:])
            mf = auxt[:, 0:DPW]
            wacc = cpool.tile([WP, DPW], f32)
            tmp = cpool.tile([WP, DPW], f32)
            nc.vector.memset(wacc[:], 0.0)
            for s in range(NUM_SHARDS):
                # tmp = (shard_map == s) * shards[s, :]
                nc.vector.scalar_tensor_tensor(
                    out=tmp[:], in0=mf, scalar=float(s),
                    in1=auxt[:, (1 + s) * DPW:(2 + s) * DPW],
                    op0=mybir.AluOpType.is_equal, op1=mybir.AluOpType.mult)
                nc.vector.tensor_add(wacc[:], wacc[:], tmp[:])
            wacch = cpool.tile([WP, DPW], bf16)
            nc.vector.tensor_copy(wacch[:], wacc[:])

            # --- flatten bf16 w onto one partition (single SB->SB DMA on
            # the ACT ring), then replicate to all 128 partitions with
            # K=1 outer-product matmuls ones[1,128].T @ wrow[1,512] ->
            # PSUM[128,512]; PE+DVE only. The flattened row is staged in
            # w128's partition 0 — the broadcast copies overwrite it with
            # identical values (1.0*w round-trips bf16->f32->bf16 exactly).
            w128 = cpool.tile([P, DIM], bf16)
            wrow = w128[0:1, :]
            nc.scalar.dma_start(wrow, wacch[:])
            MMF = 512  # one PSUM bank per matmul
            with tc.tile_pool(name="ps", bufs=8, space="PSUM") as ppool:
                for k in range(DIM // MMF):
                    mm = ppool.tile([P, MMF], f32)
                    nc.tensor.matmul(mm[:], ones[:],
                                     w128[0:1, k * MMF:(k + 1) * MMF],
                                     start=True, stop=True)
                    nc.vector.tensor_copy(w128[:, k * MMF:(k + 1) * MMF],
                                          mm[:])

            # --- stream x through SBUF, scaling by w ---
            # [128, 8192] bf16 tiles: each partition holds two consecutive
            # x rows (16KB contiguous per partition). Column half t of a
            # tile is row 2p+t, so each half multiplies against w128
            # directly. First tile runs in quarter chunks so the store
            # stream starts early; last tile stores in quarters to
            # shorten the final mul->store chain.
            x2v = x_in.rearrange("(i p t) d -> i p (t d)", p=P, t=2)
            o2v = out.rearrange("(i p t) d -> i p (t d)", p=P, t=2)
            N_BIG = ROWS_PER_CORE // (2 * P)   # 8
            QW = DIM // 2                      # quarter width (2048)
            for i in range(N_BIG):
                xt = xpool.tile([P, 2 * DIM], bf16)
                if i == 0:
                    for q in range(4):
                        cols = slice(q * QW, (q + 1) * QW)
                        nc.sync.dma_start(xt[:, cols], x2v[i, :, cols])
                        nc.vector.tensor_mul(
                            xt[:, cols], xt[:, cols],
                            w128[:, (q % 2) * QW:(q % 2 + 1) * QW])
                        nc.scalar.dma_start(o2v[i, :, cols], xt[:, cols])
                else:
                    nc.sync.dma_start(xt[:], x2v[i])
                    for h in range(2):
                        cols = slice(h * DIM, (h + 1) * DIM)
                        nc.vector.tensor_mul(xt[:, cols], xt[:, cols],
                                             w128[:])
                    if i < N_BIG - 1:
                        nc.scalar.dma_start(o2v[i], xt[:])
                    else:
                        for q in range(4):
                            cols = slice(q * QW, (q + 1) * QW)
                            nc.scalar.dma_start(o2v[i, :, cols],
                                                xt[:, cols])
    # TRN2 allows one sync wait per instruction; split multi-wait
    # instructions the way bacc's compile pipeline does.
    _bass_rust.generate_event_semaphores(nc)
    return nc


def _marshal(shards: np.ndarray, shard_map: np.ndarray):
    sh = np.asarray(shards, dtype=np.float32)
    aux = np.empty((WP, AUX_W), dtype=np.float32)
    aux[:, 0:DPW] = np.asarray(shard_map).astype(np.float32).reshape(WP, DPW)
    # aux[p, (1+s)*DPW + j] = shards[s, p*DPW + j]
    aux[:, DPW:] = sh.reshape(NUM_SHARDS, WP, DPW).transpose(
        1, 0, 2).reshape(WP, NUM_SHARDS * DPW)
    return aux


def kernel(x, shards, shard_map):
    global _cached_nc, LAST_RESULT
    if _cached_nc is None:
        _cached_nc = _build_program()
    nc = _cached_nc

    x2 = np.asarray(x, dtype=np.float32).reshape(ROWS_TOTAL, DIM).astype(BF16)
    aux = _marshal(shards, shard_map)

    in_maps = [
        {"x": x2[c * ROWS_PER_CORE:(c + 1) * ROWS_PER_CORE], "aux": aux}
        for c in range(N_CORES)
    ]
    res = run_bass_kernel_spmd(nc, in_maps, core_ids=list(range(N_CORES)),
                               trace=TRACE)
    LAST_RESULT = res
    out = np.concatenate([r["out"] for r in res.results], axis=0)
    return out.astype(np.float32).reshape(BATCH, SEQ, DIM)


# revision 3
# speedup vs baseline: 1.5746x; 1.0331x over previous
"""Distributed memory-shard scale kernel for Trainium2 (8 NeuronCores).

Computes out[b, s, d] = x[b, s, d] * shards[shard_map[d], d] for
x: [4, 4096, 4096] f32, shards: [8, 4096] f32, shard_map: [4096] int.

Strategy: data-parallel over the flattened (batch*seq) rows — each of the
8 cores owns a contiguous 2048-row slice of x and replicates the tiny
shards/shard_map inputs. The kernel is HBM/SBUF-fabric-bandwidth-bound
(pure elementwise scale), so the x stream is staged in bf16: the host
casts x to bf16, the device streams bf16 tiles, multiplies by the bf16
weight row on DVE (2x 16-bit throughput), stores bf16, and the host
upcasts to f32. This halves DMA traffic (64MB -> 32MB per core) for a
~2.9e-3 relative error, inside the 2e-2 budget.

On device each core:
  1. loads aux[s, :] = [shard_map - s | shards[s]] (bf16, 8 partitions)
     via the otherwise-idle GPSIMD SWDGE ring so the x-load ring starts
     streaming at t=0,
  2. builds the masked products B[s, d] = (shard_map[d]==s) * shards[s,d]
     with ONE fused scalar_tensor_tensor, then reduces over shards AND
     broadcasts to all 128 partitions in one step: matmul
     ones[8,128].T @ B[8,512-chunk] -> PSUM[128,512] (the mask makes each
     column's sum collapse to the single selected shard weight). The
     PSUM->SBUF bf16 casts alternate DVE/ACT so w chunks appear early,
  3. streams its x slice through SBUF in [128, 2*4096] bf16 tiles,
     multiplying each row-half by w on DVE and storing on the ACT HWDGE
     ring. First tile runs in quarter-width chunks so the store stream
     starts as early as possible; the last tile muls+stores in quarters
     to shorten the final drain.
"""

import numpy as np
import ml_dtypes

import bass_rust as _bass_rust
import concourse.bass as bass
import concourse.tile as tile
from concourse import mybir
from concourse.bass_utils import run_bass_kernel_spmd

N_CORES = 8
BATCH, SEQ, DIM = 4, 4096, 4096
NUM_SHARDS = 8
ROWS_TOTAL = BATCH * SEQ               # 16384
ROWS_PER_CORE = ROWS_TOTAL // N_CORES  # 2048
P = 128                                # SBUF partitions

BF16 = ml_dtypes.bfloat16

TRACE = False       # set True (e.g. from test.py) to capture an NTFF profile
LAST_RESULT = None  # BassKernelResults of the most recent kernel() call

_cached_nc = None


def _build_program() -> bass.Bass:
    f32 = mybir.dt.float32
    bf16 = mybir.dt.bfloat16
    nc = bass.Bass()
    x_in = nc.dram_tensor("x", [ROWS_PER_CORE, DIM], bf16, kind="ExternalInput")
    # aux[s, 0:DIM]     = shard_map - s   (bf16-exact: values in [-7, 7])
    # aux[s, DIM:2*DIM] = shards[s, :]
    aux_in = nc.dram_tensor("aux", [NUM_SHARDS, 2 * DIM], bf16,
                            kind="ExternalInput")
    out = nc.dram_tensor("out", [ROWS_PER_CORE, DIM], bf16,
                         kind="ExternalOutput")

    with tile.TileContext(nc) as tc:
        with tc.tile_pool(name="const", bufs=1) as cpool, \
             tc.tile_pool(name="xp", bufs=8) as xpool:
            # aux rides the idle GPSIMD SWDGE ring: the sync HWDGE ring
            # starts streaming x immediately, stores own the ACT ring.
            auxt = cpool.tile([NUM_SHARDS, 2 * DIM], bf16)
            nc.gpsimd.dma_start(auxt[:], aux_in[:])
            ones8 = cpool.tile([NUM_SHARDS, P], bf16)
            nc.vector.memset(ones8[:], 1.0)

            # B[s, d] = (shard_map[d] - s == 0) * shards[s, d], in place
            # over the shard_map half of aux.
            nc.vector.scalar_tensor_tensor(
                out=auxt[:, 0:DIM], in0=auxt[:, 0:DIM], scalar=0.0,
                in1=auxt[:, DIM:2 * DIM],
                op0=mybir.AluOpType.is_equal, op1=mybir.AluOpType.mult)

            # w[d] = sum_s B[s, d], replicated to 128 partitions by the
            # ones[8,128] stationary: PSUM[p, d] = sum_s ones[s,p]*B[s,d].
            # Casts alternate DVE/ACT so early w chunks unblock the first
            # tile's quarter muls while later chunks still land.
            w128 = cpool.tile([P, DIM], bf16)
            MMF = 512  # one PSUM bank per matmul
            with tc.tile_pool(name="ps", bufs=8, space="PSUM") as ppool:
                for k in range(DIM // MMF):
                    mm = ppool.tile([P, MMF], f32)
                    nc.tensor.matmul(mm[:], ones8[:],
                                     auxt[:, k * MMF:(k + 1) * MMF],
                                     start=True, stop=True)
                    eng = nc.vector if k % 2 == 0 else nc.scalar
                    if eng is nc.vector:
                        eng.tensor_copy(w128[:, k * MMF:(k + 1) * MMF],
                                        mm[:])
                    else:
                        eng.copy(w128[:, k * MMF:(k + 1) * MMF], mm[:])

            # --- stream x through SBUF, scaling by w ---
            # [128, 8192] bf16 tiles: each partition holds two consecutive
            # x rows (16KB contiguous per partition). Column half h of a
            # tile is row 2p+h, so each half multiplies against w128
            # directly. First tile runs in quarter chunks so the store
            # stream starts early; last tile muls+stores in quarters to
            # shorten the final mul->store chain.
            x2v = x_in.rearrange("(i p t) d -> i p (t d)", p=P, t=2)
            o2v = out.rearrange("(i p t) d -> i p (t d)", p=P, t=2)
            N_BIG = ROWS_PER_CORE // (2 * P)   # 8
            QW = DIM // 2                      # quarter width (2048)
            for i in range(N_BIG):
                xt = xpool.tile([P, 2 * DIM], bf16)
                if i == 0 or i == N_BIG - 1:
                    if i == 0:
                        for q in range(4):
                            cols = slice(q * QW, (q + 1) * QW)
                            nc.sync.dma_start(xt[:, cols], x2v[i, :, cols])
                    else:
                        nc.sync.dma_start(xt[:], x2v[i])
                    for q in range(4):
                        cols = slice(q * QW, (q + 1) * QW)
                        nc.vector.tensor_mul(
                            xt[:, cols], xt[:, cols],
                            w128[:, (q % 2) * QW:(q % 2 + 1) * QW])
                        nc.scalar.dma_start(o2v[i, :, cols], xt[:, cols])
                else:
                    nc.sync.dma_start(xt[:], x2v[i])
                    for h in range(2):
                        cols = slice(h * DIM, (h + 1) * DIM)
                        nc.vector.tensor_mul(xt[:, cols], xt[:, cols],
                                             w128[:])
                    nc.scalar.dma_start(o2v[i], xt[:])
    # TRN2 allows one sync wait per instruction; split multi-wait
    # instructions the way bacc's compile pipeline does.
    _bass_rust.generate_event_semaphores(nc)
    return nc


def _marshal(shards: np.ndarray, shard_map: np.ndarray):
    sh = np.asarray(shards, dtype=np.float32)
    sm = np.asarray(shard_map).astype(np.float32)
    aux = np.empty((NUM_SHARDS, 2 * DIM), dtype=BF16)
    aux[:, 0:DIM] = (sm[None, :]
                     - np.arange(NUM_SHARDS, dtype=np.float32)[:, None]
                     ).astype(BF16)
    aux[:, DIM:] = sh.astype(BF16)
    return aux


def kernel(x, shards, shard_map):
    global _cached_nc, LAST_RESULT
    if _cached_nc is None:
        _cached_nc = _build_program()
    nc = _cached_nc

    x2 = np.asarray(x, dtype=np.float32).reshape(ROWS_TOTAL, DIM).astype(BF16)
    aux = _marshal(shards, shard_map)

    in_maps = [
        {"x": x2[c * ROWS_PER_CORE:(c + 1) * ROWS_PER_CORE], "aux": aux}
        for c in range(N_CORES)
    ]
    res = run_bass_kernel_spmd(nc, in_maps, core_ids=list(range(N_CORES)),
                               trace=TRACE)
    LAST_RESULT = res
    out = np.concatenate([r["out"] for r in res.results], axis=0)
    return out.astype(np.float32).reshape(BATCH, SEQ, DIM)
